# revision 11
# baseline (speedup 1.0000x reference)
"""Trainium2 Bass kernel for nn_MessageGcn (GNN message passing).

out = relu( segsum_{recv}(x[send] @ W_f) + segsum_{send}(x[recv] @ W_b)
            + (x @ W_s) * dropout_mask )

Strategy (8 NeuronCores, SPMD, one shared program):
  - Algebraic reorder: aggregate raw x rows per destination FIRST, then apply
    the [128,128] weights once per destination node (6x less GEMM work):
        out[n] = relu( accF[n]@W_f + accB[n]@W_b + (x[n]@W_s)*mask[n] )
  - Shard destination nodes across 8 cores (12500 each); x replicated.
  - Host routing: each edge yields contributions (src=send,dst=recv,type=F)
    and (src=recv,dst=send,type=B), bucketed by (core, dst_tile_of_128,
    src_subtable_of_32768) and padded to chunks of 128 slots. One shared
    program: per-bucket chunk counts are the max over cores.
  - Device: batched dma_gather (int16 idx, 4 sub-tables) fetches ~1024
    source rows per instruction on 4 SWDGE queues. Within each group of
    dst tiles, all gathers are issued first; then each tile's segment-sum
    matmuls run consecutively: TensorE one-hot matmuls accumulate acc^T in
    PSUM (one-hot built on VectorE by comparing local-dst codes to an iota
    row; padding rows carry code -1 so their one-hot row is zero).
    Self-loop rows stream sequentially and fold in via an identity matmul.
    Per destination tile: W_f/W_b GEMMs + masked W_s GEMM, dropout mask on
    VectorE, relu on ScalarE, DMA out transposed.
"""

import numpy as np

import concourse.bass as bass
import concourse.bacc as bacc
import concourse.mybir as mybir
import concourse.tile as tile
from concourse.bass_utils import run_bass_kernel_spmd
from concourse.masks import make_identity

N = 100000
E = 600000
D = 128
P = 128
NCORES = 8
SHARD = N // NCORES           # 12500 dst nodes per core
TILES = (SHARD + P - 1) // P  # 98 dst tiles per core
SHARD_PAD = TILES * P         # 12544
KEEP_PROB = 0.8
SUB = 32768                   # subtable rows (int16 index range)
NSUB = 4                      # ceil(100001 / 32768)
GROUP = 4                     # dst tiles per gather group
MAXCH = 8                     # max chunks per dma_gather instruction
NQ = 4                        # SWDGE queues


def _route(senders, receivers):
    """Bucket contributions by (core, tile, type, subtable); build the shared
    chunk schedule and per-core index/one-hot-code arrays."""
    s = senders.astype(np.int64)
    r = receivers.astype(np.int64)
    src = np.concatenate([s, r]).astype(np.int32)
    dst = np.concatenate([r, s]).astype(np.int32)
    typ = np.concatenate([np.zeros(E, np.int8), np.ones(E, np.int8)]).astype(np.int64)

    core = dst // SHARD
    ldst_all = dst - core * SHARD
    tile_id = (ldst_all // P).astype(np.int64)
    lcol = (ldst_all % P).astype(np.float32)
    sub = (src // SUB).astype(np.int64)

    key = ((core * TILES + tile_id) * 2 + typ) * NSUB + sub
    ngroups = NCORES * TILES * 2 * NSUB
    counts = np.bincount(key, minlength=ngroups).reshape(NCORES, TILES, 2, NSUB)
    sched = (-(-counts // P)).max(axis=0)  # [TILES, 2, NSUB]

    # Stream order (gather order): for tile-group g: for subtable s:
    #   for type t: for tile in group: sched[tile, t, s] chunks
    ngrp = -(-TILES // GROUP)
    chunk_base = np.zeros((TILES, 2, NSUB), np.int64)
    nch = 0
    for g in range(ngrp):
        tl = list(range(g * GROUP, min((g + 1) * GROUP, TILES)))
        for sb in range(NSUB):
            for ty in range(2):
                for t in tl:
                    chunk_base[t, ty, sb] = nch
                    nch += int(sched[t, ty, sb])

    order = np.argsort(key, kind="stable")
    key_s = key[order]
    grp_start = np.concatenate(
        [[0], np.cumsum(np.bincount(key_s, minlength=ngroups))[:-1]])
    rank = np.arange(src.size) - grp_start[key_s]
    slot = (chunk_base[tile_id[order], typ[order], sub[order]] * P + rank)

    nslots = nch * P
    idx_flat = np.zeros((NCORES, nslots), np.int16)   # pad -> local row 0
    ldst_flat = np.full((NCORES, nslots), -1.0, np.float32)
    idx_flat[core[order], slot] = (src[order] - sub[order] * SUB).astype(np.int16)
    ldst_flat[core[order], slot] = lcol[order]

    # idx16: global slot j at [j%16 (+16*grp), j//16] — valid for any
    # instruction span starting on a chunk boundary.
    idx16 = np.zeros((NCORES, P, nslots // 16), np.int16)
    w = np.swapaxes(idx_flat.reshape(NCORES, nslots // 16, 16), 1, 2)
    for gix in range(8):
        idx16[:, gix * 16:(gix + 1) * 16, :] = w
    ldst = np.swapaxes(ldst_flat.reshape(NCORES, nch, P), 1, 2)
    return sched, chunk_base, idx16, np.ascontiguousarray(ldst), nch


def _build(sched, chunk_base, nch):
    nc = bacc.Bacc(None, target_bir_lowering=False, num_swdge_queues=NQ)
    xt = nc.dram_tensor("xt", [N + 1, D], mybir.dt.float32, kind="ExternalInput")
    wf = nc.dram_tensor("wf", [D, D], mybir.dt.float32, kind="ExternalInput")
    wb = nc.dram_tensor("wb", [D, D], mybir.dt.float32, kind="ExternalInput")
    ws = nc.dram_tensor("ws", [D, D], mybir.dt.float32, kind="ExternalInput")
    dut = nc.dram_tensor("dut", [P, SHARD_PAD], mybir.dt.float32, kind="ExternalInput")
    idx16 = nc.dram_tensor("idx16", [P, nch * 8], mybir.dt.int16, kind="ExternalInput")
    ldst = nc.dram_tensor("ldst", [P, nch], mybir.dt.float32, kind="ExternalInput")
    xown = nc.dram_tensor("xown", [SHARD_PAD, D], mybir.dt.float32, kind="ExternalInput")
    outT = nc.dram_tensor("outT", [P, SHARD_PAD], mybir.dt.float32, kind="ExternalOutput")

    qn = [0]

    with tile.TileContext(nc) as tc:
        with (
            tc.tile_pool(name="cst", bufs=1) as cst,
            tc.tile_pool(name="stage", bufs=14) as stage,
            tc.tile_pool(name="ohp", bufs=6) as ohp,
            tc.tile_pool(name="selfp", bufs=3) as selfp,
            tc.tile_pool(name="accp", bufs=3) as accp,
            tc.tile_pool(name="outp", bufs=3) as outp,
            tc.tile_pool(name="psA", bufs=3, space="PSUM") as psA,
            tc.tile_pool(name="psB", bufs=3, space="PSUM") as psB,
        ):
            iota = cst.tile([P, P], mybir.dt.float32)
            nc.gpsimd.iota(iota[:], [[1, P]], channel_multiplier=0,
                           allow_small_or_imprecise_dtypes=True)
            ident = cst.tile([P, P], mybir.dt.float32)
            make_identity(nc, ident[:])
            zeros = cst.tile([P, P], mybir.dt.float32)
            nc.vector.memset(zeros[:], 0.0)
            wf_t = cst.tile([P, D], mybir.dt.float32)
            nc.sync.dma_start(out=wf_t[:], in_=wf[:])
            wb_t = cst.tile([P, D], mybir.dt.float32)
            nc.sync.dma_start(out=wb_t[:], in_=wb[:])
            ws_t = cst.tile([P, D], mybir.dt.float32)
            nc.sync.dma_start(out=ws_t[:], in_=ws[:])
            idx_t = cst.tile([P, nch * 8], mybir.dt.int16)
            nc.sync.dma_start(out=idx_t[:], in_=idx16[:])
            ldst_t = cst.tile([P, nch], mybir.dt.float32)
            nc.sync.dma_start(out=ldst_t[:], in_=ldst[:])

            ngrp = -(-TILES // GROUP)

            def gather_span(c0, nchk, base_tbl):
                g = stage.tile([P, MAXCH * P], mybir.dt.float32, tag="g",
                               name=f"g{c0}")
                nidx = nchk * P
                nrows = min(SUB, N + 1 - base_tbl)
                nc.gpsimd.dma_gather(
                    g[:, :nchk * P].rearrange("p (c d) -> p c d", d=D),
                    xt[base_tbl:base_tbl + nrows, :],
                    idx_t[:, c0 * 8:c0 * 8 + nidx // 16],
                    nidx, nidx, D,
                    queue_num=qn[0] % NQ,
                )
                qn[0] += 1
                return g

            for g in range(ngrp):
                tl = list(range(g * GROUP, min((g + 1) * GROUP, TILES)))
                # 1) issue every gather for this group; map chunk -> slice
                gmap = {}
                for sb in range(NSUB):
                    spans = [(int(chunk_base[t, ty, sb]), int(sched[t, ty, sb]))
                             for ty in range(2) for t in tl
                             if sched[t, ty, sb] > 0]
                    if not spans:
                        continue
                    spans.sort()
                    c0 = spans[0][0]
                    cend = spans[-1][0] + spans[-1][1]
                    cc = c0
                    while cc < cend:
                        n = min(MAXCH, cend - cc)
                        gt = gather_span(cc, n, sb * SUB)
                        for k in range(n):
                            gmap[cc + k] = (gt, k)
                        cc += n
                # 2) per tile: consecutive segment-sum matmuls, then phase B
                for t in tl:
                    seg = psA.tile([P, 512], mybir.dt.float32, tag="seg",
                                   name=f"seg{t}")
                    for ty in range(2):
                        cks = [ci
                               for sb in range(NSUB)
                               for ci in range(int(chunk_base[t, ty, sb]),
                                               int(chunk_base[t, ty, sb])
                                               + int(sched[t, ty, sb]))]
                        if not cks:
                            gs0 = selfp.tile([P, D], mybir.dt.float32, tag="gs",
                                             name=f"gz{t}")
                            nc.sync.dma_start(out=gs0[:],
                                              in_=xown[t * P:(t + 1) * P, :])
                            nc.tensor.matmul(out=seg[:, ty * P:(ty + 1) * P],
                                             lhsT=gs0[:], rhs=zeros[:],
                                             start=True, stop=True)
                            continue
                        for j, ci in enumerate(cks):
                            gt, k = gmap[ci]
                            oh = ohp.tile([P, P], mybir.dt.float32, tag="oh",
                                          name=f"oh{ci}")
                            nc.vector.tensor_tensor(
                                out=oh[:],
                                in0=ldst_t[:, ci:ci + 1].to_broadcast([P, P]),
                                in1=iota[:],
                                op=mybir.AluOpType.is_equal,
                            )
                            nc.tensor.matmul(
                                out=seg[:, ty * P:(ty + 1) * P],
                                lhsT=gt[:, k * P:(k + 1) * P], rhs=oh[:],
                                start=(j == 0), stop=(j == len(cks) - 1),
                            )
                    gs = selfp.tile([P, D], mybir.dt.float32, tag="gs",
                                    name=f"gs{t}")
                    nc.sync.dma_start(out=gs[:], in_=xown[t * P:(t + 1) * P, :])
                    nc.tensor.matmul(out=seg[:, 2 * P:3 * P], lhsT=gs[:],
                                     rhs=ident[:], start=True, stop=True)

                    accT = accp.tile([P, 3 * P], mybir.dt.float32, tag="accT",
                                     name=f"accT{t}")
                    nc.scalar.copy(out=accT[:], in_=seg[:, 0:3 * P])

                    gem = psB.tile([P, 256], mybir.dt.float32, tag="gem",
                                   name=f"gem{t}")
                    nc.tensor.matmul(out=gem[:, 0:P], lhsT=wf_t[:],
                                     rhs=accT[:, 0:P], start=True, stop=False)
                    nc.tensor.matmul(out=gem[:, 0:P], lhsT=wb_t[:],
                                     rhs=accT[:, P:2 * P], start=False, stop=True)
                    nc.tensor.matmul(out=gem[:, P:2 * P], lhsT=ws_t[:],
                                     rhs=accT[:, 2 * P:3 * P], start=True, stop=True)

                    du = outp.tile([P, P], mybir.dt.float32, tag="du",
                                   name=f"du{t}")
                    nc.sync.dma_start(out=du[:], in_=dut[:, t * P:(t + 1) * P])
                    m = outp.tile([P, P], mybir.dt.float32, tag="m",
                                  name=f"m{t}")
                    nc.vector.tensor_scalar(
                        out=m[:], in0=du[:], scalar1=KEEP_PROB,
                        scalar2=1.0 / KEEP_PROB,
                        op0=mybir.AluOpType.is_lt, op1=mybir.AluOpType.mult,
                    )
                    sm = outp.tile([P, P], mybir.dt.float32, tag="sm",
                                   name=f"sm{t}")
                    nc.vector.tensor_tensor(out=sm[:], in0=gem[:, P:2 * P],
                                            in1=m[:], op=mybir.AluOpType.mult)
                    tot = outp.tile([P, P], mybir.dt.float32, tag="tot",
                                    name=f"tot{t}")
                    nc.vector.tensor_tensor(out=tot[:], in0=gem[:, 0:P],
                                            in1=sm[:], op=mybir.AluOpType.add)
                    ot = outp.tile([P, P], mybir.dt.float32, tag="ot",
                                   name=f"ot{t}")
                    nc.scalar.activation(out=ot[:], in_=tot[:],
                                         func=mybir.ActivationFunctionType.Relu)
                    nc.sync.dma_start(out=outT[:, t * P:(t + 1) * P], in_=ot[:])
    nc.compile()
    return nc


def kernel(x, W_f, W_b, W_s, drop_u, senders, receivers):
    x = np.asarray(x, np.float32)
    W_f = np.asarray(W_f, np.float32)
    W_b = np.asarray(W_b, np.float32)
    W_s = np.asarray(W_s, np.float32)
    drop_u = np.asarray(drop_u, np.float32)

    sched, chunk_base, idx16, ldst, nch = _route(np.asarray(senders),
                                                 np.asarray(receivers))
    nc = _build(sched, chunk_base, nch)

    xt = np.concatenate([x, np.zeros((1, D), np.float32)], axis=0)
    in_maps = []
    for c in range(NCORES):
        lo = c * SHARD
        du = np.zeros((SHARD_PAD, D), np.float32)
        du[:SHARD] = drop_u[lo:lo + SHARD]
        xo = np.zeros((SHARD_PAD, D), np.float32)
        xo[:SHARD] = x[lo:lo + SHARD]
        in_maps.append({
            "xt": xt, "wf": W_f, "wb": W_b, "ws": W_s,
            "dut": np.ascontiguousarray(du.T),
            "idx16": np.ascontiguousarray(idx16[c]),
            "ldst": np.ascontiguousarray(ldst[c]),
            "xown": xo,
        })

    res = run_bass_kernel_spmd(nc, in_maps, core_ids=list(range(NCORES)))
    out = np.empty((N, D), np.float32)
    for c in range(NCORES):
        out[c * SHARD:(c + 1) * SHARD] = res.results[c]["outT"][:, :SHARD].T
    return out


# revision 12
# speedup vs baseline: 1.5345x; 1.5345x over previous
"""Trainium2 Bass kernel for nn_MessageGcn (GNN message passing).

out = relu( segsum_{recv}(x[send] @ W_f) + segsum_{send}(x[recv] @ W_b)
            + (x @ W_s) * dropout_mask )

Strategy (8 NeuronCores, SPMD, one shared program):
  - Algebraic reorder: aggregate raw x rows per destination FIRST, then apply
    the [128,128] weights once per destination node:
        out[n] = relu( accF[n]@W_f + accB[n]@W_b + (x[n]@W_s)*mask[n] )
    where accF[n] = sum_{e: recv[e]=n} x[send[e]],
          accB[n] = sum_{e: send[e]=n} x[recv[e]].
    This cuts GEMM work 6x vs edge-space GEMMs.
  - Shard destination nodes across 8 cores (12500 nodes each); x is
    replicated so each core gathers source rows locally.
  - Host routing: each edge contributes (src=send, dst=recv, type=F) and
    (src=recv, dst=send, type=B). Contributions are bucketed by
    (core, dst_tile_of_128) and padded to chunks of 128. All cores share one
    compiled program, so per-(type,tile) chunk counts are the max over cores;
    padding rows point at a zeros row of the table with local-dst -1.
  - Device: indirect DMA gathers 128 source rows per chunk into SBUF;
    TensorE computes acc^T[tile] via one-hot matmul
    (acc^T = gathered^T-free: out[feat, dst] = sum_p g[p,feat]*onehot[p,dst])
    accumulating chunks in PSUM; the one-hot is built on VectorE by
    comparing local-dst codes against an iota row. Self-loop rows are
    streamed sequentially and folded in with an identity matrix.
    Then per destination tile: W_f/W_b GEMMs (+ masked W_s GEMM), dropout
    mask from drop_u on VectorE, relu on ScalarE, DMA out (transposed).
"""

import numpy as np

import concourse.bass as bass
import concourse.bacc as bacc
import concourse.mybir as mybir
import concourse.tile as tile
from concourse.bass_utils import run_bass_kernel_spmd
from concourse.masks import make_identity

N = 100000
E = 600000
D = 128
P = 128
NCORES = 8
SHARD = N // NCORES          # 12500 dst nodes per core
TILES = (SHARD + P - 1) // P  # 98 dst tiles per core
SHARD_PAD = TILES * P         # 12544
KEEP_PROB = 0.8
ZROW = N                      # index of the appended zeros row in the table


def _route(senders, receivers):
    """Build per-core gather/onehot metadata. Returns (sched, gidx, ldst):
    sched[t] = (nf, nb) chunks for tile t (shared across cores);
    gidx[c]  = int32 [P, NCH] source-row index per (chunk, slot);
    ldst[c]  = float32 [P, NCH] local dst (0..127) or -1 for padding.
    Chunk order: tile 0 (F chunks.. B chunks), tile 1 (...), ...
    """
    s = senders.astype(np.int64)
    r = receivers.astype(np.int64)
    # contributions: type F: (src=s, dst=r); type B: (src=r, dst=s)
    src = np.concatenate([s, r]).astype(np.int32)
    dst = np.concatenate([r, s]).astype(np.int32)
    typ = np.concatenate([np.zeros(E, np.int8), np.ones(E, np.int8)])

    core = dst // SHARD
    ldst_all = dst - core * SHARD
    tile_id = ldst_all // P
    lcol = (ldst_all % P).astype(np.float32)

    # group key: (core, tile, type)
    key = (core.astype(np.int64) * TILES + tile_id) * 2 + typ
    ngroups = NCORES * TILES * 2
    counts = np.bincount(key, minlength=ngroups).reshape(NCORES, TILES, 2)
    chunks = -(-counts // P)  # ceil
    sched_ft = chunks.max(axis=0)  # [TILES, 2] shared schedule

    # chunk-slot base offset of each (core, tile, type) group in the stream
    per_tile = sched_ft.sum(axis=1)          # chunks per tile
    tile_base = np.concatenate([[0], np.cumsum(per_tile)[:-1]])  # chunk idx
    nch = int(per_tile.sum())
    # group slot base in "slot" units (slot = chunk*P + partition)
    grp_base = np.empty((TILES, 2), np.int64)
    grp_base[:, 0] = tile_base * P
    grp_base[:, 1] = (tile_base + sched_ft[:, 0]) * P

    order = np.argsort(key, kind="stable")
    key_sorted = key[order]
    # rank within group
    grp_start_pos = np.concatenate([[0], np.cumsum(np.bincount(key_sorted, minlength=ngroups))[:-1]])
    rank = np.arange(src.size) - grp_start_pos[key_sorted]

    core_s = core[order]
    tile_s = tile_id[order]
    typ_s = typ[order].astype(np.int64)
    slot = grp_base[tile_s, typ_s] + rank

    gidx = np.full((NCORES, P, nch), ZROW, np.int32)
    ldst = np.full((NCORES, P, nch), -1.0, np.float32)
    gidx[core_s, slot % P, slot // P] = src[order]
    ldst[core_s, slot % P, slot // P] = lcol[order]
    return sched_ft, gidx, ldst, nch


def _build(sched_ft, nch):
    nc = bacc.Bacc(None, target_bir_lowering=False)
    xt = nc.dram_tensor("xt", [N + 1, D], mybir.dt.float32, kind="ExternalInput")
    wf = nc.dram_tensor("wf", [D, D], mybir.dt.float32, kind="ExternalInput")
    wb = nc.dram_tensor("wb", [D, D], mybir.dt.float32, kind="ExternalInput")
    ws = nc.dram_tensor("ws", [D, D], mybir.dt.float32, kind="ExternalInput")
    dut = nc.dram_tensor("dut", [P, SHARD_PAD], mybir.dt.float32, kind="ExternalInput")
    gidx = nc.dram_tensor("gidx", [P, nch], mybir.dt.int32, kind="ExternalInput")
    ldst = nc.dram_tensor("ldst", [P, nch], mybir.dt.float32, kind="ExternalInput")
    xown = nc.dram_tensor("xown", [SHARD_PAD, D], mybir.dt.float32, kind="ExternalInput")
    outT = nc.dram_tensor("outT", [P, SHARD_PAD], mybir.dt.float32, kind="ExternalOutput")

    with tile.TileContext(nc) as tc:
        with (
            tc.tile_pool(name="cst", bufs=1) as cst,
            tc.tile_pool(name="stage", bufs=8) as stage,
            tc.tile_pool(name="ohp", bufs=8) as ohp,
            tc.tile_pool(name="accp", bufs=3) as accp,
            tc.tile_pool(name="outp", bufs=3) as outp,
            tc.tile_pool(name="psA", bufs=3, space="PSUM") as psA,
            tc.tile_pool(name="psB", bufs=3, space="PSUM") as psB,
        ):
            iota = cst.tile([P, P], mybir.dt.float32)
            nc.gpsimd.iota(iota[:], [[1, P]], channel_multiplier=0,
                           allow_small_or_imprecise_dtypes=True)
            ident = cst.tile([P, P], mybir.dt.float32)
            make_identity(nc, ident[:])
            wf_t = cst.tile([P, D], mybir.dt.float32)
            nc.sync.dma_start(out=wf_t[:], in_=wf[:])
            wb_t = cst.tile([P, D], mybir.dt.float32)
            nc.sync.dma_start(out=wb_t[:], in_=wb[:])
            ws_t = cst.tile([P, D], mybir.dt.float32)
            nc.sync.dma_start(out=ws_t[:], in_=ws[:])
            gidx_t = cst.tile([P, nch], mybir.dt.int32)
            nc.sync.dma_start(out=gidx_t[:], in_=gidx[:])
            ldst_t = cst.tile([P, nch], mybir.dt.float32)
            nc.sync.dma_start(out=ldst_t[:], in_=ldst[:])

            ci = 0
            for t in range(TILES):
                nf, nb = int(sched_ft[t, 0]), int(sched_ft[t, 1])
                seg = psA.tile([P, 512], mybir.dt.float32, tag="seg")
                # F chunks -> seg[:, 0:128]; B chunks -> seg[:, 128:256];
                # self -> seg[:, 256:384]
                for typi, ntyp in ((0, nf), (1, nb)):
                    for k in range(ntyp):
                        g = stage.tile([P, D], mybir.dt.float32, tag="g")
                        nc.gpsimd.indirect_dma_start(
                            out=g[:], out_offset=None, in_=xt[:],
                            in_offset=bass.IndirectOffsetOnAxis(
                                ap=gidx_t[:, ci:ci + 1], axis=0),
                        )
                        oh = ohp.tile([P, P], mybir.dt.float32, tag="oh")
                        nc.vector.tensor_tensor(
                            out=oh[:],
                            in0=ldst_t[:, ci:ci + 1].to_broadcast([P, P]),
                            in1=iota[:],
                            op=mybir.AluOpType.is_equal,
                        )
                        nc.tensor.matmul(
                            out=seg[:, typi * P:(typi + 1) * P],
                            lhsT=g[:], rhs=oh[:],
                            start=(k == 0), stop=(k == ntyp - 1),
                        )
                        ci += 1
                # self rows: sequential stream + identity matmul
                gs = stage.tile([P, D], mybir.dt.float32, tag="g")
                nc.sync.dma_start(out=gs[:], in_=xown[t * P:(t + 1) * P, :])
                nc.tensor.matmul(out=seg[:, 2 * P:3 * P], lhsT=gs[:],
                                 rhs=ident[:], start=True, stop=True)

                accT = accp.tile([P, 3 * P], mybir.dt.float32, tag="accT")
                nc.scalar.copy(out=accT[:], in_=seg[:, 0:3 * P])

                gem = psB.tile([P, 256], mybir.dt.float32, tag="gem")
                nc.tensor.matmul(out=gem[:, 0:P], lhsT=wf_t[:],
                                 rhs=accT[:, 0:P], start=True, stop=False)
                nc.tensor.matmul(out=gem[:, 0:P], lhsT=wb_t[:],
                                 rhs=accT[:, P:2 * P], start=False, stop=True)
                nc.tensor.matmul(out=gem[:, P:2 * P], lhsT=ws_t[:],
                                 rhs=accT[:, 2 * P:3 * P], start=True, stop=True)

                du = outp.tile([P, P], mybir.dt.float32, tag="du")
                nc.sync.dma_start(out=du[:], in_=dut[:, t * P:(t + 1) * P])
                m = outp.tile([P, P], mybir.dt.float32, tag="m")
                nc.vector.tensor_scalar(
                    out=m[:], in0=du[:], scalar1=KEEP_PROB,
                    scalar2=1.0 / KEEP_PROB,
                    op0=mybir.AluOpType.is_lt, op1=mybir.AluOpType.mult,
                )
                sm = outp.tile([P, P], mybir.dt.float32, tag="sm")
                nc.vector.tensor_tensor(out=sm[:], in0=gem[:, P:2 * P],
                                        in1=m[:], op=mybir.AluOpType.mult)
                tot = outp.tile([P, P], mybir.dt.float32, tag="tot")
                nc.vector.tensor_tensor(out=tot[:], in0=gem[:, 0:P],
                                        in1=sm[:], op=mybir.AluOpType.add)
                ot = outp.tile([P, P], mybir.dt.float32, tag="ot")
                nc.scalar.activation(out=ot[:], in_=tot[:],
                                     func=mybir.ActivationFunctionType.Relu)
                nc.sync.dma_start(out=outT[:, t * P:(t + 1) * P], in_=ot[:])
    nc.compile()
    return nc


def kernel(x, W_f, W_b, W_s, drop_u, senders, receivers):
    x = np.asarray(x, np.float32)
    W_f = np.asarray(W_f, np.float32)
    W_b = np.asarray(W_b, np.float32)
    W_s = np.asarray(W_s, np.float32)
    drop_u = np.asarray(drop_u, np.float32)
    senders = np.asarray(senders)
    receivers = np.asarray(receivers)

    sched_ft, gidx, ldst, nch = _route(senders, receivers)
    nc = _build(sched_ft, nch)

    xt = np.concatenate([x, np.zeros((1, D), np.float32)], axis=0)
    in_maps = []
    for c in range(NCORES):
        lo = c * SHARD
        du = np.zeros((SHARD_PAD, D), np.float32)
        du[:SHARD] = drop_u[lo:lo + SHARD]
        xo = np.zeros((SHARD_PAD, D), np.float32)
        xo[:SHARD] = x[lo:lo + SHARD]
        in_maps.append({
            "xt": xt, "wf": W_f, "wb": W_b, "ws": W_s,
            "dut": np.ascontiguousarray(du.T),
            "gidx": np.ascontiguousarray(gidx[c]),
            "ldst": np.ascontiguousarray(ldst[c]),
            "xown": xo,
        })

    res = run_bass_kernel_spmd(nc, in_maps, core_ids=list(range(NCORES)))
    out = np.empty((N, D), np.float32)
    for c in range(NCORES):
        out[c * SHARD:(c + 1) * SHARD] = res.results[c]["outT"][:, :SHARD].T
    return out
